# revision 11
# baseline (speedup 1.0000x reference)
"""Trainium2 Bass kernel for nn_DTransformer_10909216932644.

Sharding: 8 cores = 4 batches x 2 sequence halves. Feature-major (transposed)
activations [D, T_local]. Attention is head-split within each pair (8 heads
per core over the full 2048-token sequence, causal blocks only), using pair
AllGather + per-core permuted QKV weights + 0/1 blend scalars so the single
SPMD program is rank-agnostic.

v1 optimizations vs baseline:
- bf16 weights for QKV/W1 (LN gains folded host-side, biases via b'=b/g in
  LN output), host pre-tiled weight layouts for contiguous DMA, each weight
  tile loaded once per layer (not once per token-half).
- Paired [128,1024] PSUM tiles: QKV gemms accumulate both token halves, the
  two attention-score matmuls per block share one tile so exp runs on
  [128,1024]; PSUM = 2 tags x 2 bufs x 2 banks = 8 banks.
- Own-Q / own-Y / h stay in SBUF (no DRAM round trips); MLP accumulates all
  32 contraction chunks in PSUM (no DVE partial adds) and fuses
  bias+residual into one scalar_tensor_tensor per output tile.
"""

import sys

sys.path.insert(0, "/opt/trn_rl_repo")

import numpy as np
import ml_dtypes

import concourse.bass as bass
import concourse.tile as tile
from concourse import bacc, mybir
from concourse.bass_utils import run_bass_kernel_spmd

F32 = mybir.dt.float32
F32R = mybir.dt.float32r
BF16 = mybir.dt.bfloat16
AF = mybir.ActivationFunctionType
OP = mybir.AluOpType

L = 8
D = 1024
H = 16
HD = 64
M = 4096
V = 64
B, T = 4, 2048
TL = 1024
EPS = 1e-5
NLH = 8
NP = 4
NC = 8
DC = 8
NQT = 4

_CACHE = {}


def build_program(n_layers=L, repeat=1, skip=()):
    nc = bacc.Bacc("TRN2", target_bir_lowering=False, debug=False, num_devices=NC)

    toksT = nc.dram_tensor("toksT", [V, TL], F32R, kind="ExternalInput")
    posT = nc.dram_tensor("posT", [D, TL], F32, kind="ExternalInput")
    wtokT = nc.dram_tensor("wtokT", [V, D], F32R, kind="ExternalInput")
    wqkh = nc.dram_tensor("wqkh", [n_layers * 16, 128, 1024], BF16, kind="ExternalInput")
    wvh = nc.dram_tensor("wvh", [n_layers * 2, 128, 4096], BF16, kind="ExternalInput")
    w1h = nc.dram_tensor("w1h", [n_layers * 16, 128, 2048], BF16, kind="ExternalInput")
    w2h = nc.dram_tensor("w2h", [n_layers * 8, 128, 4096], BF16, kind="ExternalInput")
    bvec = nc.dram_tensor("bvec", [2 * n_layers + 1, D], F32, kind="ExternalInput")
    wunT = nc.dram_tensor("wunT", [D, V], BF16, kind="ExternalInput")
    bm1_in = nc.dram_tensor("bm1", [n_layers, M], F32, kind="ExternalInput")
    bm2_in = nc.dram_tensor("bm2", [n_layers, D], F32, kind="ExternalInput")
    bun = nc.dram_tensor("bun", [V, 1], F32, kind="ExternalInput")
    masks_in = nc.dram_tensor("masks", [4, 128, 512], BF16, kind="ExternalInput")
    sel2_in = nc.dram_tensor("sel2", [65, 128], F32R, kind="ExternalInput")
    ones_in = nc.dram_tensor("ones", [128, 128], F32R, kind="ExternalInput")
    uv_in = nc.dram_tensor("uv", [128, 3], F32, kind="ExternalInput")

    outT = nc.dram_tensor("outT", [V, TL], F32, kind="ExternalOutput")

    with tile.TileContext(nc) as tc:
        with (
            tc.tile_pool(name="per", bufs=1) as per,
            tc.tile_pool(name="xtp", bufs=1) as xtp,
            tc.tile_pool(name="big", bufs=1) as big,
            tc.tile_pool(name="kv", bufs=1) as kv,
            tc.tile_pool(name="att", bufs=1) as att,
            tc.tile_pool(name="stg", bufs=2) as stg,
            tc.tile_pool(name="strm", bufs=2) as strm,
            tc.tile_pool(name="sml", bufs=2) as sml,
            tc.tile_pool(name="wp", bufs=2) as wp,
            tc.tile_pool(name="esp", bufs=2) as esp,
            tc.tile_pool(name="ps_mm", bufs=2, space="PSUM") as ps_mm,
            tc.tile_pool(name="ps_pv", bufs=2, space="PSUM") as ps_pv,
            tc.tile_pool(name="dram", bufs=2, space="DRAM") as dram,
        ):
            xT = xtp.tile([128, DC * TL], F32R, tag="xT")
            consts = per.tile([128, 128], F32R, tag="ones")
            nc.sync.dma_start(consts[:], ones_in.ap())
            sel2 = per.tile([65, 128], F32R, tag="sel2")
            nc.sync.dma_start(sel2[:], sel2_in.ap())
            rcp65 = per.tile([65, 512], F32R, tag="rcp65")
            nc.vector.memset(rcp65[:], 0.0)
            uvw = per.tile([128, 3], F32, tag="uvw")
            nc.sync.dma_start(uvw[:], uv_in.ap())
            onebf = per.tile([128, 1], BF16, tag="onebf")
            nc.vector.memset(onebf[:], 1.0)
            mask_sb = per.tile([128, 4 * 512], BF16, tag="masks")
            nc.sync.dma_start(
                mask_sb[:].rearrange("p (m t) -> p m t", m=4),
                masks_in.ap().rearrange("m p t -> p m t"),
            )
            ones_col = consts[:, 0:1]
            ones_row = consts[0:1, :]
            uvec = uvw[:, 0:1]
            eps_ap = uvw[0:1, 2:3]
            wvec = uvw[:, 1:2]

            def coef(h):
                return uvec if h == 0 else wvec

            def coefo(h):
                return wvec if h == 0 else uvec

            def ln_apply(src_fn, row, src_16=False, xn_tag="xn"):
                """LN over features (partition dim) via ones-matmul.
                src_fn(dc, tt) -> SBUF AP [128, 512].
                xn = (x - mu) * rstd + b'  (gains folded into weights host-side;
                b' = b/g).  Returns xn tile [128, 8192] bf16."""
                b_all = sml.tile([128, 8], F32, tag="b_all")
                nc.sync.dma_start(
                    b_all[:],
                    bvec.ap()[row : row + 1, :].rearrange("o (c p) -> (o p) c", p=128),
                )
                tA = sml.tile([1, TL], F32, tag="lnA", bufs=1)
                tB = sml.tile([1, TL], F32, tag="lnB", bufs=1)
                rstd_t = sml.tile([1, TL], F32R, tag="rstd", bufs=1)
                bb_t = sml.tile([1, TL], F32R, tag="bbt", bufs=1)
                oc_ = onebf[:] if src_16 else ones_col
                sq_dt = BF16 if src_16 else F32R
                for tt in range(2):
                    s_pair = ps_mm.tile([128, 1024], F32, tag="mm")
                    s1 = s_pair[0:1, 0:512]
                    s2 = s_pair[0:1, 512:1024]
                    for dc in range(DC):
                        sl = src_fn(dc, tt)
                        nc.tensor.matmul(
                            s1, oc_, sl, start=(dc == 0), stop=(dc == DC - 1)
                        )
                        sq = stg.tile([128, 512], sq_dt, tag="sq", bufs=2)
                        nc.scalar.activation(sq[:], sl, AF.Square, scale=1.0)
                        nc.tensor.matmul(
                            s2, oc_, sq[:], start=(dc == 0), stop=(dc == DC - 1)
                        )
                    nc.scalar.copy(tA[0:1, tt * 512 : tt * 512 + 512], s1)
                    nc.scalar.copy(tB[0:1, tt * 512 : tt * 512 + 512], s2)
                # tA: sx -> mu ; tB: sxx -> msq -> var -> se
                nc.vector.tensor_scalar(tA[:], tA[:], 1.0 / D, None, op0=OP.mult)
                nc.vector.tensor_scalar(tB[:], tB[:], 1.0 / D, None, op0=OP.mult)
                with nc.allow_low_precision(reason="f32r musq"):
                    nc.vector.tensor_tensor(rstd_t[:], tA[:], tA[:], op=OP.mult)
                nc.vector.tensor_tensor(tB[:], tB[:], rstd_t[:], op=OP.subtract)
                nc.scalar.activation(tB[:], tB[:], AF.Sqrt, bias=eps_ap, scale=1.0)
                with nc.allow_low_precision(reason="f32r rstd"):
                    nc.vector.reciprocal(rstd_t[:], tB[:])
                nc.vector.scalar_tensor_tensor(
                    bb_t[:], tA[:], -1.0, rstd_t[:], op0=OP.mult, op1=OP.mult
                )
                xn = big.tile([128, DC * TL], BF16, tag=xn_tag)
                for tt in range(2):
                    ab = ps_mm.tile([128, 1024], F32, tag="mm")
                    nc.tensor.matmul(
                        ab[:, 0:512], ones_row, rstd_t[0:1, tt * 512 : tt * 512 + 512],
                        start=True, stop=True,
                    )
                    nc.tensor.matmul(
                        ab[:, 512:1024], ones_row, bb_t[0:1, tt * 512 : tt * 512 + 512],
                        start=True, stop=True,
                    )
                    for dc in range(DC):
                        sl = src_fn(dc, tt)
                        u1 = stg.tile([128, 512], BF16, tag="u1", bufs=2)
                        nc.vector.tensor_tensor(u1[:], sl, ab[:, 0:512], op=OP.mult)
                        nc.vector.scalar_tensor_tensor(
                            xn[:, dc * TL + tt * 512 : dc * TL + tt * 512 + 512],
                            u1[:],
                            b_all[:, dc : dc + 1],
                            ab[:, 512:1024],
                            op0=OP.add,
                            op1=OP.add,
                        )
                return xn

            def x_slice(dc, tt):
                return xT[:, dc * TL + tt * 512 : dc * TL + tt * 512 + 512]

            # ---------------- embed ----------------
            tok_sb = att.tile([V, TL], F32R, tag="qown")
            nc.sync.dma_start(tok_sb[:], toksT.ap())
            for dc in range(DC):
                wte = wp.tile([128, 128], F32R, tag="wte")
                nc.sync.dma_start(wte[0:64, :], wtokT.ap()[:, dc * 128 : (dc + 1) * 128])
                e_ps = ps_mm.tile([128, 1024], F32, tag="mm")
                for tt in range(2):
                    nc.tensor.matmul(
                        e_ps[:, tt * 512 : tt * 512 + 512], wte[0:64, :],
                        tok_sb[:, tt * 512 : tt * 512 + 512],
                        start=True, stop=True,
                    )
                for tt in range(2):
                    pc = stg.tile([128, 512], F32, tag="u1")
                    nc.sync.dma_start(
                        pc[:], posT.ap()[dc * 128 : dc * 128 + 128, tt * 512 : tt * 512 + 512]
                    )
                    sl = slice(dc * TL + tt * 512, dc * TL + tt * 512 + 512)
                    nc.vector.tensor_tensor(
                        xT[:, sl], e_ps[:, tt * 512 : tt * 512 + 512], pc[:], op=OP.add
                    )

            # ---------------- layers ----------------
            for rep in range(repeat):
              if rep > 0:  # timing-only variant: keep values bounded
                for c in range(DC):
                    nc.vector.tensor_scalar(
                        xT[:, c * TL : (c + 1) * TL], xT[:, c * TL : (c + 1) * TL],
                        0.05, None, op0=OP.mult,
                    )
              for l in range(n_layers):
                xn = ln_apply(x_slice, 2 * l, src_16=False, xn_tag="xn")

                ag1_in = dram.tile([1536, TL], BF16, tag="ag1_in")
                ag1_out = dram.tile([2 * 1536, TL], BF16, tag="ag1_out")
                ag2_in = dram.tile([512, TL], BF16, tag="ag2_in")
                ag2_out = dram.tile([1024, TL], BF16, tag="ag2_out")

                qown = att.tile([128, NP * TL], BF16, tag="qown")
                KT = kv.tile([128, NP * T], BF16, tag="KT")
                Vsb = kv.tile([128, 16 * NLH * (HD + 1) + 128], BF16, tag="Vsb")
                vs4 = Vsb[:, 0 : 16 * NLH * (HD + 1)].rearrange(
                    "p (k h c) -> p k h c", k=16, c=HD + 1
                )
                nc.vector.memset(vs4[:, :, :, HD : HD + 1], 1.0)

                def xn_sl(dc, tt, _xn=xn):
                    return _xn[:, dc * TL + tt * 512 : dc * TL + tt * 512 + 512]

                def gemm_pair(out_ps, wt, rhs_fn):
                    """out_ps [128,1024] <- both token halves, contracting DC chunks."""
                    for dc in range(DC):
                        w_sl = wt[:, dc * 128 : (dc + 1) * 128]
                        nc.tensor.matmul(
                            out_ps[:, 0:512], w_sl, rhs_fn(dc, 0),
                            start=(dc == 0), stop=(dc == DC - 1),
                        )
                        nc.tensor.matmul(
                            out_ps[:, 512:1024], w_sl, rhs_fn(dc, 1),
                            start=(dc == 0), stop=(dc == DC - 1),
                        )

                # --- Q projection ---
                for oc in range(8 if "qkv" not in skip else 0):
                    wt = wp.tile([128, 1024], BF16, tag="wt")
                    nc.scalar.dma_start(wt[:], wqkh.ap()[l * 16 + oc])
                    q_ps = ps_mm.tile([128, 1024], F32, tag="mm")
                    gemm_pair(q_ps, wt, xn_sl)
                    if oc < 4:
                        nc.scalar.copy(qown[:, oc * TL : (oc + 1) * TL], q_ps[:])
                    else:
                        st = stg.tile([128, 1024], BF16, tag="qstg")
                        nc.scalar.copy(st[:], q_ps[:])
                        nc.sync.dma_start(
                            ag1_in[128 * (oc - 4) : 128 * (oc - 4) + 128, :], st[:]
                        )
                # --- K projection ---
                for oc in range(8 if "qkv" not in skip else 0):
                    wt = wp.tile([128, 1024], BF16, tag="wt")
                    nc.scalar.dma_start(wt[:], wqkh.ap()[l * 16 + 8 + oc])
                    k_ps = ps_mm.tile([128, 1024], F32, tag="mm")
                    gemm_pair(k_ps, wt, xn_sl)
                    if oc < 4:
                        for h in range(2):
                            nc.vector.tensor_scalar(
                                KT[:, 2048 * oc + 1024 * h : 2048 * oc + 1024 * h + 1024],
                                k_ps[:], coef(h), None, op0=OP.mult,
                            )
                    else:
                        st = stg.tile([128, 1024], BF16, tag="qstg")
                        nc.scalar.copy(st[:], k_ps[:])
                        nc.sync.dma_start(
                            ag1_in[512 + 128 * (oc - 4) : 512 + 128 * (oc - 4) + 128, :],
                            st[:],
                        )
                # --- V projection (natural layout): lhsT = xn chunk, rhs = w ---
                for oh in range(2 if "qkv" not in skip else 0):
                    wtv = wp.tile([128, 4096], BF16, tag="wtv", bufs=1)
                    nc.scalar.dma_start(wtv[:], wvh.ap()[l * 2 + oh])
                    for tvp in range(4):
                        v_ps = ps_mm.tile([128, 1024], F32, tag="mm")
                        for dc in range(DC):
                            for tv2 in range(2):
                                tv = 2 * tvp + tv2
                                nc.tensor.matmul(
                                    v_ps[:, tv2 * 512 : tv2 * 512 + 512],
                                    xn[:, dc * TL + tv * 128 : dc * TL + tv * 128 + 128],
                                    wtv[:, dc * 512 : dc * 512 + 512],
                                    start=(dc == 0),
                                    stop=(dc == DC - 1),
                                )
                        for tv2 in range(2):
                            tv = 2 * tvp + tv2
                            vr = v_ps[:, tv2 * 512 : tv2 * 512 + 512].rearrange(
                                "p (h c) -> p h c", h=NLH
                            )
                            if oh == 0:
                                for h in range(2):
                                    nc.vector.tensor_scalar(
                                        vs4[:, 8 * h + tv, :, 0:HD], vr, coef(h), None,
                                        op0=OP.mult,
                                    )
                            else:
                                st = stg.tile([128, 512], BF16, tag="qstg")
                                nc.scalar.copy(st[:], v_ps[:, tv2 * 512 : tv2 * 512 + 512])
                                vsec = ag1_in[1024:1536, :].rearrange(
                                    "a (b c) -> (a b) c", b=2
                                )
                                nc.sync.dma_start(
                                    vsec[tv * 128 : tv * 128 + 128, :], st[:]
                                )

                if "coll" not in skip:
                    nc.gpsimd.collective_compute(
                        "AllGather", OP.bypass,
                        replica_groups=[[0, 1], [2, 3], [4, 5], [6, 7]],
                        ins=[ag1_in.opt()], outs=[ag1_out.opt()],
                    )

                # --- K/V assembly pass 2 (blob adds) ---
                for h in range(2 if "asm" not in skip else 0):
                    blob = 1536 * h
                    for p in range(NP):
                        kb = strm.tile([128, TL], BF16, tag="kb", bufs=1)
                        nc.sync.dma_start(
                            kb[:],
                            ag1_out[blob + 512 + 128 * p : blob + 512 + 128 * p + 128, :],
                        )
                        dsl = KT[:, 2048 * p + 1024 * h : 2048 * p + 1024 * h + 1024]
                        nc.vector.scalar_tensor_tensor(
                            dsl, kb[:], coefo(h), dsl, op0=OP.mult, op1=OP.add
                        )
                    vsec = ag1_out[blob + 1024 : blob + 1536, :].rearrange(
                        "a (b c) -> (a b) c", b=2
                    )
                    for kl in range(8):
                        vb = strm.tile([128, 512], BF16, tag="vb", bufs=1)
                        nc.sync.dma_start(vb[:], vsec[kl * 128 : kl * 128 + 128, :])
                        vbr = vb[:].rearrange("p (h c) -> p h c", h=NLH)
                        dst = vs4[:, 8 * h + kl, :, 0:HD]
                        nc.vector.scalar_tensor_tensor(
                            dst, vbr, coefo(h), dst, op0=OP.mult, op1=OP.add
                        )

                # --- attention ---
                yown = att.tile([128, NP * T], BF16, tag="yown")
                qor = qown[:].rearrange("p (g t) -> p g t", g=NP)
                for q in range(NQT if "attn" not in skip else 0):
                    half = q // 2
                    qb = strm.tile([128, NP * 512], BF16, tag="qb", bufs=1)
                    nc.sync.dma_start(
                        qb[:].rearrange("p (g t) -> p g t", g=NP),
                        ag1_out[1536 * half : 1536 * half + 512,
                                512 * (q % 2) : 512 * (q % 2) + 512].rearrange(
                            "(g p) t -> p g t", p=128
                        ),
                    )
                    qt_t = strm.tile([128, NP * 512], BF16, tag="qt", bufs=2)
                    qtr = qt_t[:].rearrange("p (g t) -> p g t", g=NP)
                    nc.vector.tensor_scalar(
                        qtr,
                        qor[:, :, 512 * (q % 2) : 512 * (q % 2) + 512],
                        coef(half), None, op0=OP.mult,
                    )
                    nc.vector.scalar_tensor_tensor(
                        qt_t[:], qb[:], coefo(half), qt_t[:], op0=OP.mult, op1=OP.add
                    )
                    for p in range(NP):
                        pv = ps_pv.tile([128, 1024], F32, tag="pv")
                        for k in range(4 * q + 4):
                            dlt = k - 4 * q
                            s_ps = ps_mm.tile([128, 1024], F32, tag="mm")
                            for e in range(2):
                                base = 64 * e
                                nc.tensor.matmul(
                                    s_ps[:, 512 * e : 512 * e + 512],
                                    KT[base : base + 64,
                                       2048 * p + 128 * k : 2048 * p + 128 * k + 128],
                                    qt_t[base : base + 64, 512 * p : 512 * p + 512],
                                    start=True, stop=True,
                                )
                            es = esp.tile([128, 1024], BF16, tag="es", bufs=2)
                            if "attn_noexp" not in skip:
                                nc.scalar.activation(es[:], s_ps[:], AF.Exp, scale=0.125)
                                if dlt >= 0:
                                    for e in range(2):
                                        nc.vector.tensor_tensor(
                                            es[:, 512 * e : 512 * e + 512],
                                            es[:, 512 * e : 512 * e + 512],
                                            mask_sb[:, 512 * dlt : 512 * dlt + 512],
                                            op=OP.mult,
                                        )
                            if "attn_nopv" in skip or "attn_noexp" in skip:
                                continue
                            for e in range(2):
                                nc.tensor.matmul(
                                    pv[:, 512 * e : 512 * e + 512],
                                    Vsb[:, 520 * k + 65 * (2 * p + e) :
                                        520 * k + 65 * (2 * p + e) + 128],
                                    es[:, 512 * e : 512 * e + 512],
                                    start=(k == 0),
                                    stop=(k == 4 * q + 3),
                                )
                        if "attn_notail" in skip or "attn_nopv" in skip or "attn_noexp" in skip:
                            continue
                        with nc.allow_low_precision(reason="f32r softmax rcp"):
                            nc.vector.reciprocal(rcp65[0:1, :], pv[64:65, 0:512])
                            nc.vector.reciprocal(rcp65[64:65, :], pv[64:65, 512:1024])
                        rcb_ps = ps_mm.tile([128, 1024], F32, tag="mm")
                        nc.tensor.matmul(
                            rcb_ps[:, 0:512], sel2[:], rcp65[:], start=True, stop=True
                        )
                        rcb = stg.tile([128, 512], BF16, tag="rcb", bufs=2)
                        nc.scalar.copy(rcb[:], rcb_ps[:, 0:512])
                        ysl = yown[:, 2048 * p + 512 * q : 2048 * p + 512 * q + 512]
                        for e in range(2):
                            nc.vector.tensor_tensor(
                                ysl[64 * e : 64 * e + 64, :],
                                pv[0:64, 512 * e : 512 * e + 512],
                                rcb[64 * e : 64 * e + 64, :],
                                op=OP.mult,
                            )

                # --- AG2: contribute my heads for partner tokens ---
                agst = att.tile([128, NP * TL], BF16, tag="qown")
                for p in range(NP):
                    csl = agst[:, p * TL : (p + 1) * TL]
                    nc.vector.tensor_scalar(
                        csl, yown[:, 2048 * p : 2048 * p + TL], wvec, None, op0=OP.mult
                    )
                    nc.vector.scalar_tensor_tensor(
                        csl, yown[:, 2048 * p + TL : 2048 * p + T], uvec, csl,
                        op0=OP.mult, op1=OP.add,
                    )
                nc.sync.dma_start(
                    ag2_in.rearrange("(g p) t -> p g t", p=128),
                    agst[:].rearrange("p (g t) -> p g t", g=NP),
                )
                if "coll" not in skip:
                    nc.gpsimd.collective_compute(
                        "AllGather", OP.bypass,
                        replica_groups=[[0, 1], [2, 3], [4, 5], [6, 7]],
                        ins=[ag2_in.opt()], outs=[ag2_out.opt()],
                    )

                # --- h assembly: h = x + y (SBUF bf16); x = 2x + y ---
                hT = kv.tile([128, DC * TL], BF16, tag="KT")
                for c in range(DC):
                    low = c < 4
                    bbc = strm.tile([128, TL], BF16, tag="bbc", bufs=2)
                    nc.sync.dma_start(
                        bbc[:],
                        ag2_out[(0 if low else 512) + 128 * (c % 4) :
                                (0 if low else 512) + 128 * (c % 4) + 128, :],
                    )
                    t1 = strm.tile([128, TL], BF16, tag="t1", bufs=1)
                    nc.vector.tensor_scalar(
                        t1[:],
                        yown[:, 2048 * (c % 4) + (0 if low else TL) :
                             2048 * (c % 4) + (TL if low else T)],
                        coef(0 if low else 1), None, op0=OP.mult,
                    )
                    nc.vector.scalar_tensor_tensor(
                        t1[:], bbc[:], coefo(0 if low else 1), t1[:],
                        op0=OP.mult, op1=OP.add,
                    )
                    xs = xT[:, c * TL : (c + 1) * TL]
                    nc.vector.tensor_tensor(
                        hT[:, c * TL : (c + 1) * TL], xs, t1[:], op=OP.add
                    )
                    nc.vector.scalar_tensor_tensor(
                        xs, xs, 2.0, t1[:], op0=OP.mult, op1=OP.add
                    )

                def h_slice(dc, tt, _h=hT):
                    return _h[:, dc * TL + tt * 512 : dc * TL + tt * 512 + 512]

                xn2 = ln_apply(h_slice, 2 * l + 1, src_16=True, xn_tag="xn")

                # --- MLP ---
                def xn2_sl(dc, tt, _x=xn2):
                    return _x[:, dc * TL + tt * 512 : dc * TL + tt * 512 + 512]

                bm1_sb = sml.tile([128, 32], F32, tag="bm1")
                nc.sync.dma_start(
                    bm1_sb[:],
                    bm1_in.ap()[l : l + 1, :].rearrange("o (c p) -> (o p) c", p=128),
                )
                bm2_sb = sml.tile([128, 8], F32, tag="bm2")
                nc.sync.dma_start(
                    bm2_sb[:],
                    bm2_in.ap()[l : l + 1, :].rearrange("o (c p) -> (o p) c", p=128),
                )
                for tt in range(2 if "mlp" not in skip else 0):
                    zA = kv.tile([128, 16 * 512], BF16, tag="KT")
                    zB = kv.tile([128, 16 * 512], BF16, tag="Vsb")

                    def zsl(j, _zA=zA, _zB=zB):
                        t_ = _zA if j < 16 else _zB
                        return t_[:, (j % 16) * 512 : (j % 16) * 512 + 512]

                    for u in range(16):
                        wt1 = wp.tile([128, 2048], BF16, tag="wt1")
                        nc.scalar.dma_start(wt1[:], w1h.ap()[l * 16 + u])
                        z_ps = ps_mm.tile([128, 1024], F32, tag="mm")
                        for dc in range(DC):
                            r_sl = xn2_sl(dc, tt)
                            for j2 in range(2):
                                nc.tensor.matmul(
                                    z_ps[:, j2 * 512 : j2 * 512 + 512],
                                    wt1[:, j2 * 1024 + dc * 128 : j2 * 1024 + dc * 128 + 128],
                                    r_sl,
                                    start=(dc == 0), stop=(dc == DC - 1),
                                )
                        for j2 in range(2):
                            j = 2 * u + j2
                            nc.scalar.activation(
                                zsl(j), z_ps[:, j2 * 512 : j2 * 512 + 512], AF.Relu,
                                bias=bm1_sb[:, j : j + 1], scale=1.0,
                            )
                    for oc in range(8):
                        wt2 = wp.tile([128, 4096], BF16, tag="wt2")
                        nc.scalar.dma_start(wt2[:], w2h.ap()[l * 8 + oc])
                        d_ps = ps_mm.tile([128, 512], F32, tag="mm")
                        for j in range(32):
                            nc.tensor.matmul(
                                d_ps[:], wt2[:, j * 128 : j * 128 + 128], zsl(j),
                                start=(j == 0), stop=(j == 31),
                            )
                        xsl = xT[:, oc * TL + tt * 512 : oc * TL + tt * 512 + 512]
                        nc.vector.scalar_tensor_tensor(
                            xsl, d_ps[:], bm2_sb[:, oc : oc + 1], xsl,
                            op0=OP.add, op1=OP.add,
                        )

            # ---------------- final LN + unembed ----------------
            xnf = ln_apply(x_slice, 2 * n_layers, src_16=False, xn_tag="xn")
            bun_sb = sml.tile([V, 1], F32, tag="bun", bufs=1)
            nc.sync.dma_start(bun_sb[:], bun.ap())
            out_sb = att.tile([V, TL], F32, tag="qown")
            for tt in range(2):
                o_ps = ps_mm.tile([64, 512], F32, tag="mm")
                for dc in range(DC):
                    wtu = wp.tile([128, 64], BF16, tag="wtu")
                    nc.sync.dma_start(wtu[:], wunT.ap()[dc * 128 : (dc + 1) * 128, :])
                    nc.tensor.matmul(
                        o_ps[:], wtu[:],
                        xnf[:, dc * TL + tt * 512 : dc * TL + tt * 512 + 512],
                        start=(dc == 0), stop=(dc == DC - 1),
                    )
                nc.scalar.activation(
                    out_sb[:, tt * 512 : tt * 512 + 512], o_ps[:], AF.Identity,
                    bias=bun_sb[:, 0:1], scale=1.0,
                )
            nc.sync.dma_start(outT.ap(), out_sb[:])

    nc.compile()
    return nc


# ---------------- host side ----------------


def prep_inputs(inputs, n_layers=L):
    f32 = np.float32
    bf16 = ml_dtypes.bfloat16
    toks = np.asarray(inputs["toks"], f32)
    W_tok = np.asarray(inputs["W_tok"], f32)
    W_pos = np.asarray(inputs["W_pos"], f32)
    Wqkv = np.asarray(inputs["Wqkv"], f32)
    W1 = np.asarray(inputs["W1"], f32)
    W2 = np.asarray(inputs["W2"], f32)
    Wun = np.asarray(inputs["Wun"], f32)
    bun = np.asarray(inputs["bun"], f32)
    g1, be1 = np.asarray(inputs["g1"], f32), np.asarray(inputs["be1"], f32)
    g2, be2 = np.asarray(inputs["g2"], f32), np.asarray(inputs["be2"], f32)
    gf, bf_ = np.asarray(inputs["gf"], f32), np.asarray(inputs["bf"], f32)
    bm1 = np.asarray(inputs["bm1"], f32)
    bm2 = np.asarray(inputs["bm2"], f32)

    # Fold LN gains into the consuming weights; biases become b' = b/g added
    # to the normalized activations inside the kernel.
    def safediv(b, g):
        return b / np.where(g == 0.0, 1.0, g)

    Wq = Wqkv[:n_layers] * g1[:n_layers, None, :]
    W1f = W1[:n_layers] * g2[:n_layers, None, :]
    Wunf = Wun * gf[None, :]
    bvec = np.zeros((2 * n_layers + 1, D), f32)
    bvec[0 : 2 * n_layers : 2] = safediv(be1[:n_layers], g1[:n_layers])
    bvec[1 : 2 * n_layers : 2] = safediv(be2[:n_layers], g2[:n_layers])
    bvec[2 * n_layers] = safediv(bf_, gf)

    masks = np.zeros((4, 128, 512), np.float32)
    i = np.arange(128)[:, None]
    j = np.arange(512)[None, :]
    for d in range(4):
        masks[d] = (j >= 128 * d + i).astype(f32)
    masks_bf = masks.astype(bf16)
    sel2 = np.zeros((65, 128), f32)
    sel2[0, 0:64] = 1.0
    sel2[64, 64:128] = 1.0
    ones = np.ones((128, 128), f32)

    # W1 lhsT j-pair tiles: w1h[l*16+u][p, j2*1024 + c*128 + o]
    #   = W1f[l, (2u+j2)*128 + o, c*128 + p]
    w1h = np.ascontiguousarray(
        W1f.reshape(n_layers, 16, 2, 128, 8, 128).transpose(0, 1, 5, 2, 4, 3)
    ).reshape(n_layers * 16, 128, 2048).astype(bf16)
    # W2 lhsT per-oc tiles: w2h[l*8+oc][p, j*128 + o] = W2[l, oc*128+o, j*128+p]
    w2h = np.ascontiguousarray(
        W2[:n_layers].reshape(n_layers, 8, 128, 32, 128).transpose(0, 1, 4, 3, 2)
    ).reshape(n_layers * 8, 128, 4096).astype(bf16)
    wunT = np.ascontiguousarray(Wunf.T).astype(bf16)
    wtokT = np.ascontiguousarray(W_tok.T)

    in_maps = []
    for c in range(NC):
        b, jj = c // 2, c % 2
        ho = list(range(8 * jj, 8 * jj + 8)) + list(range(8 * (1 - jj), 8 * (1 - jj) + 8))
        idx_q = np.concatenate([np.arange(192 * h, 192 * h + 64) for h in ho])
        perm = np.concatenate([idx_q, idx_q + 64, idx_q + 128])
        Wqp = Wq[:, perm, :]
        # Q/K lhsT tiles: wqkh[l*16+oc][p, c*128+o] = Wqp[l, oc*128+o, c*128+p]
        wqkh = np.ascontiguousarray(
            Wqp[:, :2048].reshape(n_layers, 16, 128, 8, 128).transpose(0, 1, 4, 3, 2)
        ).reshape(n_layers * 16, 128, 1024).astype(bf16)
        # V rhs tiles: wvh[l*2+oh][p, c*512+o] = Wqp[l, 2048 + oh*512 + o, c*128+p]
        wvh = np.ascontiguousarray(
            Wqp[:, 2048:].reshape(n_layers, 2, 512, 8, 128).transpose(0, 1, 4, 3, 2)
        ).reshape(n_layers * 2, 128, 4096).astype(bf16)
        u = 1.0 if jj == 0 else 0.0
        uv = np.zeros((128, 3), f32)
        uv[:, 0] = u
        uv[:, 1] = 1.0 - u
        uv[:, 2] = EPS
        in_maps.append(
            {
                "toksT": np.ascontiguousarray(toks[b, TL * jj : TL * jj + TL, :].T),
                "posT": np.ascontiguousarray(W_pos[:, TL * jj : TL * jj + TL]),
                "wtokT": wtokT,
                "wqkh": wqkh,
                "wvh": wvh,
                "w1h": w1h,
                "w2h": w2h,
                "bvec": bvec,
                "wunT": wunT,
                "bm1": bm1[:n_layers],
                "bm2": bm2[:n_layers],
                "bun": bun.reshape(V, 1),
                "masks": masks_bf,
                "sel2": sel2,
                "ones": ones,
                "uv": uv,
            }
        )
    return in_maps


def kernel(**inputs):
    if "prog" not in _CACHE:
        _CACHE["prog"] = build_program()
    nc = _CACHE["prog"]
    in_maps = prep_inputs(inputs)
    res = run_bass_kernel_spmd(nc, in_maps, list(range(NC)))
    out = np.zeros((B, T, V), np.float32)
    for c in range(NC):
        b, jj = c // 2, c % 2
        out[b, TL * jj : TL * jj + TL, :] = res.results[c]["outT"].T
    return out


# revision 13
# speedup vs baseline: 2.3164x; 2.3164x over previous
"""Trainium2 Bass kernel for nn_DTransformer_10909216932644.

Sharding: 8 cores = 4 batches x 2 sequence halves. Feature-major (transposed)
activations [D, T_local]. Attention is head-split within each pair (8 heads
per core over the full 2048-token sequence, causal blocks only), using pair
AllGather + per-core permuted QKV weights + 0/1 blend scalars so the single
SPMD program is rank-agnostic.

v1 optimizations vs baseline:
- bf16 weights for QKV/W1 (LN gains folded host-side, biases via b'=b/g in
  LN output), host pre-tiled weight layouts for contiguous DMA, each weight
  tile loaded once per layer (not once per token-half).
- Paired [128,1024] PSUM tiles: QKV gemms accumulate both token halves, the
  two attention-score matmuls per block share one tile so exp runs on
  [128,1024]; PSUM = 2 tags x 2 bufs x 2 banks = 8 banks.
- Own-Q / own-Y / h stay in SBUF (no DRAM round trips); MLP accumulates all
  32 contraction chunks in PSUM (no DVE partial adds) and fuses
  bias+residual into one scalar_tensor_tensor per output tile.
"""

import sys

sys.path.insert(0, "/opt/trn_rl_repo")

import numpy as np
import ml_dtypes

import concourse.bass as bass
import concourse.tile as tile
from concourse import bacc, mybir
from concourse.bass_utils import run_bass_kernel_spmd

F32 = mybir.dt.float32
F32R = mybir.dt.float32r
BF16 = mybir.dt.bfloat16
AF = mybir.ActivationFunctionType
OP = mybir.AluOpType

L = 8
D = 1024
H = 16
HD = 64
M = 4096
V = 64
B, T = 4, 2048
TL = 1024
EPS = 1e-5
NLH = 8
NP = 4
NC = 8
DC = 8
NQT = 4

_CACHE = {}


def build_program(n_layers=L, repeat=1, skip=()):
    nc = bacc.Bacc("TRN2", target_bir_lowering=False, debug=False, num_devices=NC)

    toksT = nc.dram_tensor("toksT", [V, TL], F32R, kind="ExternalInput")
    posT = nc.dram_tensor("posT", [D, TL], F32, kind="ExternalInput")
    wtokT = nc.dram_tensor("wtokT", [V, D], F32R, kind="ExternalInput")
    wqkh = nc.dram_tensor("wqkh", [n_layers * 16, 128, 1024], BF16, kind="ExternalInput")
    wvh = nc.dram_tensor("wvh", [n_layers * 2, 128, 4096], BF16, kind="ExternalInput")
    w1h = nc.dram_tensor("w1h", [n_layers * 16, 128, 2048], BF16, kind="ExternalInput")
    w2h = nc.dram_tensor("w2h", [n_layers * 8, 128, 4096], BF16, kind="ExternalInput")
    bvec = nc.dram_tensor("bvec", [2 * n_layers + 1, D], F32, kind="ExternalInput")
    wunT = nc.dram_tensor("wunT", [D, V], BF16, kind="ExternalInput")
    bm1_in = nc.dram_tensor("bm1", [n_layers, M], F32, kind="ExternalInput")
    bm2_in = nc.dram_tensor("bm2", [n_layers, D], F32, kind="ExternalInput")
    bun = nc.dram_tensor("bun", [V, 1], F32, kind="ExternalInput")
    masks_in = nc.dram_tensor("masks", [4, 128, 512], BF16, kind="ExternalInput")
    sel2_in = nc.dram_tensor("sel2", [65, 128], F32R, kind="ExternalInput")
    ones_in = nc.dram_tensor("ones", [128, 128], F32R, kind="ExternalInput")
    uv_in = nc.dram_tensor("uv", [128, 3], F32, kind="ExternalInput")

    outT = nc.dram_tensor("outT", [V, TL], F32, kind="ExternalOutput")

    with tile.TileContext(nc) as tc:
        with (
            tc.tile_pool(name="per", bufs=1) as per,
            tc.tile_pool(name="xtp", bufs=1) as xtp,
            tc.tile_pool(name="big", bufs=1) as big,
            tc.tile_pool(name="kv", bufs=1) as kv,
            tc.tile_pool(name="att", bufs=1) as att,
            tc.tile_pool(name="stg", bufs=2) as stg,
            tc.tile_pool(name="strm", bufs=2) as strm,
            tc.tile_pool(name="sml", bufs=2) as sml,
            tc.tile_pool(name="wp", bufs=2) as wp,
            tc.tile_pool(name="esp", bufs=2) as esp,
            tc.tile_pool(name="ps_mm", bufs=2, space="PSUM") as ps_mm,
            tc.tile_pool(name="ps_pv", bufs=2, space="PSUM") as ps_pv,
            tc.tile_pool(name="dram", bufs=2, space="DRAM") as dram,
        ):
            xT = xtp.tile([128, DC * TL], F32R, tag="xT")
            consts = per.tile([128, 128], F32R, tag="ones")
            nc.sync.dma_start(consts[:], ones_in.ap())
            sel2 = per.tile([65, 128], F32R, tag="sel2")
            nc.sync.dma_start(sel2[:], sel2_in.ap())
            rcp65 = per.tile([65, 512], F32R, tag="rcp65")
            nc.vector.memset(rcp65[:].bitcast(F32), 0.0)
            uvw = per.tile([128, 3], F32, tag="uvw")
            nc.sync.dma_start(uvw[:], uv_in.ap())
            onebf = per.tile([128, 1], BF16, tag="onebf")
            nc.vector.memset(onebf[:], 1.0)
            mask_sb = per.tile([128, 4 * 512], BF16, tag="masks")
            nc.sync.dma_start(
                mask_sb[:].rearrange("p (m t) -> p m t", m=4),
                masks_in.ap().rearrange("m p t -> p m t"),
            )
            ones_col = consts[:, 0:1]
            ones_row = consts[0:1, :]
            uvec = uvw[:, 0:1]
            eps_ap = uvw[0:1, 2:3]
            wvec = uvw[:, 1:2]

            def coef(h):
                return uvec if h == 0 else wvec

            def coefo(h):
                return wvec if h == 0 else uvec

            def ln_apply(src_fn, row, src_16=False, xn_tag="xn"):
                """LN over features (partition dim) via ones-matmul.
                src_fn(dc, tt) -> SBUF AP [128, 512].
                xn = (x - mu) * rstd + b'  (gains folded into weights host-side;
                b' = b/g).  Returns xn tile [128, 8192] bf16."""
                b_all = sml.tile([128, 8], F32, tag="b_all")
                nc.sync.dma_start(
                    b_all[:],
                    bvec.ap()[row : row + 1, :].rearrange("o (c p) -> (o p) c", p=128),
                )
                tA = sml.tile([1, TL], F32, tag="lnA", bufs=1)
                tB = sml.tile([1, TL], F32, tag="lnB", bufs=1)
                rstd_t = sml.tile([1, TL], F32R, tag="rstd", bufs=1)
                bb_t = sml.tile([1, TL], F32R, tag="bbt", bufs=1)
                oc_ = onebf[:] if src_16 else ones_col
                sq_dt = BF16 if src_16 else F32R
                for tt in range(2):
                    s_pair = ps_mm.tile([128, 1024], F32, tag="mm")
                    s1 = s_pair[0:1, 0:512]
                    s2 = s_pair[0:1, 512:1024]
                    for dc in range(DC):
                        sl = src_fn(dc, tt)
                        nc.tensor.matmul(
                            s1, oc_, sl, start=(dc == 0), stop=(dc == DC - 1)
                        )
                        sq = stg.tile([128, 512], sq_dt, tag="sq", bufs=2)
                        nc.scalar.activation(sq[:], sl, AF.Square, scale=1.0)
                        nc.tensor.matmul(
                            s2, oc_, sq[:], start=(dc == 0), stop=(dc == DC - 1)
                        )
                    nc.scalar.copy(tA[0:1, tt * 512 : tt * 512 + 512], s1)
                    nc.scalar.copy(tB[0:1, tt * 512 : tt * 512 + 512], s2)
                # tA: sx -> mu ; tB: sxx -> msq -> var -> se
                nc.vector.tensor_scalar(tA[:], tA[:], 1.0 / D, None, op0=OP.mult)
                nc.vector.tensor_scalar(tB[:], tB[:], 1.0 / D, None, op0=OP.mult)
                with nc.allow_low_precision(reason="f32r musq"):
                    nc.vector.tensor_tensor(rstd_t[:], tA[:], tA[:], op=OP.mult)
                nc.vector.tensor_tensor(tB[:], tB[:], rstd_t[:], op=OP.subtract)
                nc.scalar.activation(tB[:], tB[:], AF.Sqrt, bias=eps_ap, scale=1.0)
                with nc.allow_low_precision(reason="f32r rstd"):
                    nc.vector.reciprocal(rstd_t[:], tB[:])
                nc.vector.scalar_tensor_tensor(
                    bb_t[:], tA[:], -1.0, rstd_t[:], op0=OP.mult, op1=OP.mult
                )
                xn = big.tile([128, DC * TL], BF16, tag=xn_tag)
                for tt in range(2):
                    ab = ps_mm.tile([128, 1024], F32, tag="mm")
                    nc.tensor.matmul(
                        ab[:, 0:512], ones_row, rstd_t[0:1, tt * 512 : tt * 512 + 512],
                        start=True, stop=True,
                    )
                    nc.tensor.matmul(
                        ab[:, 512:1024], ones_row, bb_t[0:1, tt * 512 : tt * 512 + 512],
                        start=True, stop=True,
                    )
                    for dc in range(DC):
                        sl = src_fn(dc, tt)
                        u1 = stg.tile([128, 512], BF16, tag="u1", bufs=2)
                        nc.vector.tensor_tensor(u1[:], sl, ab[:, 0:512], op=OP.mult)
                        nc.vector.scalar_tensor_tensor(
                            xn[:, dc * TL + tt * 512 : dc * TL + tt * 512 + 512],
                            u1[:],
                            b_all[:, dc : dc + 1],
                            ab[:, 512:1024],
                            op0=OP.add,
                            op1=OP.add,
                        )
                return xn

            def x_slice(dc, tt):
                return xT[:, dc * TL + tt * 512 : dc * TL + tt * 512 + 512]

            # ---------------- embed ----------------
            tok_sb = att.tile([V, TL], F32R, tag="qown")
            nc.sync.dma_start(tok_sb[:], toksT.ap())
            for dc in range(DC):
                wte = wp.tile([128, 128], F32R, tag="wte")
                nc.sync.dma_start(wte[0:64, :], wtokT.ap()[:, dc * 128 : (dc + 1) * 128])
                e_ps = ps_mm.tile([128, 1024], F32, tag="mm")
                for tt in range(2):
                    nc.tensor.matmul(
                        e_ps[:, tt * 512 : tt * 512 + 512], wte[0:64, :],
                        tok_sb[:, tt * 512 : tt * 512 + 512],
                        start=True, stop=True,
                    )
                for tt in range(2):
                    pc = stg.tile([128, 512], F32, tag="u1")
                    nc.sync.dma_start(
                        pc[:], posT.ap()[dc * 128 : dc * 128 + 128, tt * 512 : tt * 512 + 512]
                    )
                    sl = slice(dc * TL + tt * 512, dc * TL + tt * 512 + 512)
                    nc.vector.tensor_tensor(
                        xT[:, sl], e_ps[:, tt * 512 : tt * 512 + 512], pc[:], op=OP.add
                    )

            # ---------------- layers ----------------
            for rep in range(repeat):
              if rep > 0:  # timing-only variant: keep values bounded
                for c in range(DC):
                    nc.vector.tensor_scalar(
                        xT[:, c * TL : (c + 1) * TL], xT[:, c * TL : (c + 1) * TL],
                        0.05, None, op0=OP.mult,
                    )
              for l in range(n_layers):
                xn = ln_apply(x_slice, 2 * l, src_16=False, xn_tag="xn")

                ag1_in = dram.tile([1536, TL], BF16, tag="ag1_in")
                ag1_out = dram.tile([2 * 1536, TL], BF16, tag="ag1_out")
                ag2_in = dram.tile([512, TL], BF16, tag="ag2_in")
                ag2_out = dram.tile([1024, TL], BF16, tag="ag2_out")

                qown = att.tile([128, NP * TL], BF16, tag="qown")
                KT = kv.tile([128, NP * T], BF16, tag="KT")
                Vsb = kv.tile([128, 16 * NLH * (HD + 1) + 128], BF16, tag="Vsb")
                vs4 = Vsb[:, 0 : 16 * NLH * (HD + 1)].rearrange(
                    "p (k h c) -> p k h c", k=16, c=HD + 1
                )
                nc.vector.memset(vs4[:, :, :, HD : HD + 1], 1.0)

                def xn_sl(dc, tt, _xn=xn):
                    return _xn[:, dc * TL + tt * 512 : dc * TL + tt * 512 + 512]

                def gemm_pair(out_ps, wt, rhs_fn):
                    """out_ps [128,1024] <- both token halves, contracting DC chunks."""
                    for dc in range(DC):
                        w_sl = wt[:, dc * 128 : (dc + 1) * 128]
                        nc.tensor.matmul(
                            out_ps[:, 0:512], w_sl, rhs_fn(dc, 0),
                            start=(dc == 0), stop=(dc == DC - 1),
                        )
                        nc.tensor.matmul(
                            out_ps[:, 512:1024], w_sl, rhs_fn(dc, 1),
                            start=(dc == 0), stop=(dc == DC - 1),
                        )

                # --- Q projection ---
                for oc in range(8 if "qkv" not in skip else 0):
                    wt = wp.tile([128, 1024], BF16, tag="wt")
                    nc.sync.dma_start(wt[:], wqkh.ap()[l * 16 + oc])
                    q_ps = ps_mm.tile([128, 1024], F32, tag="mm")
                    gemm_pair(q_ps, wt, xn_sl)
                    if oc < 4:
                        nc.scalar.copy(qown[:, oc * TL : (oc + 1) * TL], q_ps[:])
                    else:
                        st = stg.tile([128, 1024], BF16, tag="qstg")
                        nc.scalar.copy(st[:], q_ps[:])
                        nc.sync.dma_start(
                            ag1_in[128 * (oc - 4) : 128 * (oc - 4) + 128, :], st[:]
                        )
                # --- K projection ---
                for oc in range(8 if "qkv" not in skip else 0):
                    wt = wp.tile([128, 1024], BF16, tag="wt")
                    nc.sync.dma_start(wt[:], wqkh.ap()[l * 16 + 8 + oc])
                    k_ps = ps_mm.tile([128, 1024], F32, tag="mm")
                    gemm_pair(k_ps, wt, xn_sl)
                    if oc < 4:
                        for h in range(2):
                            nc.vector.tensor_scalar(
                                KT[:, 2048 * oc + 1024 * h : 2048 * oc + 1024 * h + 1024],
                                k_ps[:], coef(h), None, op0=OP.mult,
                            )
                    else:
                        st = stg.tile([128, 1024], BF16, tag="qstg")
                        nc.scalar.copy(st[:], k_ps[:])
                        nc.sync.dma_start(
                            ag1_in[512 + 128 * (oc - 4) : 512 + 128 * (oc - 4) + 128, :],
                            st[:],
                        )
                # --- V projection (natural layout): lhsT = xn chunk, rhs = w ---
                for oh in range(2 if "qkv" not in skip else 0):
                    wtv = wp.tile([128, 4096], BF16, tag="wtv", bufs=1)
                    nc.sync.dma_start(wtv[:], wvh.ap()[l * 2 + oh])
                    for tvp in range(4):
                        v_ps = ps_mm.tile([128, 1024], F32, tag="mm")
                        for dc in range(DC):
                            for tv2 in range(2):
                                tv = 2 * tvp + tv2
                                nc.tensor.matmul(
                                    v_ps[:, tv2 * 512 : tv2 * 512 + 512],
                                    xn[:, dc * TL + tv * 128 : dc * TL + tv * 128 + 128],
                                    wtv[:, dc * 512 : dc * 512 + 512],
                                    start=(dc == 0),
                                    stop=(dc == DC - 1),
                                )
                        for tv2 in range(2):
                            tv = 2 * tvp + tv2
                            vr = v_ps[:, tv2 * 512 : tv2 * 512 + 512].rearrange(
                                "p (h c) -> p h c", h=NLH
                            )
                            if oh == 0:
                                for h in range(2):
                                    nc.vector.tensor_scalar(
                                        vs4[:, 8 * h + tv, :, 0:HD], vr, coef(h), None,
                                        op0=OP.mult,
                                    )
                            else:
                                st = stg.tile([128, 512], BF16, tag="qstg")
                                nc.scalar.copy(st[:], v_ps[:, tv2 * 512 : tv2 * 512 + 512])
                                vsec = ag1_in[1024:1536, :].rearrange(
                                    "a (b c) -> (a b) c", b=2
                                )
                                nc.sync.dma_start(
                                    vsec[tv * 128 : tv * 128 + 128, :], st[:]
                                )

                if "coll" not in skip:
                    nc.gpsimd.collective_compute(
                        "AllGather", OP.bypass,
                        replica_groups=[[0, 1], [2, 3], [4, 5], [6, 7]],
                        ins=[ag1_in.opt()], outs=[ag1_out.opt()],
                    )

                # --- K/V assembly pass 2 (blob adds) ---
                for h in range(2 if "asm" not in skip else 0):
                    blob = 1536 * h
                    for p in range(NP):
                        kb = strm.tile([128, TL], BF16, tag="kb", bufs=1)
                        nc.sync.dma_start(
                            kb[:],
                            ag1_out[blob + 512 + 128 * p : blob + 512 + 128 * p + 128, :],
                        )
                        dsl = KT[:, 2048 * p + 1024 * h : 2048 * p + 1024 * h + 1024]
                        nc.vector.scalar_tensor_tensor(
                            dsl, kb[:], coefo(h), dsl, op0=OP.mult, op1=OP.add
                        )
                    vsec = ag1_out[blob + 1024 : blob + 1536, :].rearrange(
                        "a (b c) -> (a b) c", b=2
                    )
                    for kl in range(8):
                        vb = strm.tile([128, 512], BF16, tag="vb", bufs=1)
                        nc.sync.dma_start(vb[:], vsec[kl * 128 : kl * 128 + 128, :])
                        vbr = vb[:].rearrange("p (h c) -> p h c", h=NLH)
                        dst = vs4[:, 8 * h + kl, :, 0:HD]
                        nc.vector.scalar_tensor_tensor(
                            dst, vbr, coefo(h), dst, op0=OP.mult, op1=OP.add
                        )

                # --- attention ---
                yown = att.tile([128, NP * T], BF16, tag="yown")
                qor = qown[:].rearrange("p (g t) -> p g t", g=NP)
                for q in range(NQT if "attn" not in skip else 0):
                    half = q // 2
                    qb = strm.tile([128, NP * 512], BF16, tag="qb", bufs=1)
                    nc.sync.dma_start(
                        qb[:].rearrange("p (g t) -> p g t", g=NP),
                        ag1_out[1536 * half : 1536 * half + 512,
                                512 * (q % 2) : 512 * (q % 2) + 512].rearrange(
                            "(g p) t -> p g t", p=128
                        ),
                    )
                    qt_t = strm.tile([128, NP * 512], BF16, tag="qt", bufs=2)
                    qtr = qt_t[:].rearrange("p (g t) -> p g t", g=NP)
                    nc.vector.tensor_scalar(
                        qtr,
                        qor[:, :, 512 * (q % 2) : 512 * (q % 2) + 512],
                        coef(half), None, op0=OP.mult,
                    )
                    nc.vector.scalar_tensor_tensor(
                        qt_t[:], qb[:], coefo(half), qt_t[:], op0=OP.mult, op1=OP.add
                    )
                    for p in range(NP):
                        pv = ps_pv.tile([128, 1024], F32, tag="pv")
                        for k in range(4 * q + 4):
                            dlt = k - 4 * q
                            s_ps = ps_mm.tile([128, 1024], F32, tag="mm")
                            for e in range(2):
                                base = 64 * e
                                nc.tensor.matmul(
                                    s_ps[:, 512 * e : 512 * e + 512],
                                    KT[base : base + 64,
                                       2048 * p + 128 * k : 2048 * p + 128 * k + 128],
                                    qt_t[base : base + 64, 512 * p : 512 * p + 512],
                                    start=True, stop=True,
                                )
                            es = esp.tile([128, 1024], BF16, tag="es", bufs=2)
                            if "attn_noexp" not in skip:
                                nc.scalar.activation(es[:], s_ps[:], AF.Exp, scale=0.125)
                                if dlt >= 0:
                                    for e in range(2):
                                        nc.vector.tensor_tensor(
                                            es[:, 512 * e : 512 * e + 512],
                                            es[:, 512 * e : 512 * e + 512],
                                            mask_sb[:, 512 * dlt : 512 * dlt + 512],
                                            op=OP.mult,
                                        )
                            if "attn_nopv" in skip or "attn_noexp" in skip:
                                continue
                            for e in range(2):
                                nc.tensor.matmul(
                                    pv[:, 512 * e : 512 * e + 512],
                                    Vsb[:, 520 * k + 65 * (2 * p + e) :
                                        520 * k + 65 * (2 * p + e) + 128],
                                    es[:, 512 * e : 512 * e + 512],
                                    start=(k == 0),
                                    stop=(k == 4 * q + 3),
                                )
                        if "attn_notail" in skip or "attn_nopv" in skip or "attn_noexp" in skip:
                            continue
                        with nc.allow_low_precision(reason="f32r softmax rcp"):
                            nc.vector.reciprocal(rcp65[0:1, :], pv[64:65, 0:512])
                            nc.vector.reciprocal(rcp65[64:65, :], pv[64:65, 512:1024])
                        rcb_ps = ps_mm.tile([128, 1024], F32, tag="mm")
                        nc.tensor.matmul(
                            rcb_ps[:, 0:512], sel2[:], rcp65[:], start=True, stop=True
                        )
                        rcb = stg.tile([128, 512], BF16, tag="rcb", bufs=2)
                        nc.scalar.copy(rcb[:], rcb_ps[:, 0:512])
                        ysl = yown[:, 2048 * p + 512 * q : 2048 * p + 512 * q + 512]
                        for e in range(2):
                            nc.vector.tensor_tensor(
                                ysl[64 * e : 64 * e + 64, :],
                                pv[0:64, 512 * e : 512 * e + 512],
                                rcb[64 * e : 64 * e + 64, :],
                                op=OP.mult,
                            )

                # --- AG2: contribute my heads for partner tokens ---
                agst = att.tile([128, NP * TL], BF16, tag="qown")
                for p in range(NP):
                    csl = agst[:, p * TL : (p + 1) * TL]
                    nc.vector.tensor_scalar(
                        csl, yown[:, 2048 * p : 2048 * p + TL], wvec, None, op0=OP.mult
                    )
                    nc.vector.scalar_tensor_tensor(
                        csl, yown[:, 2048 * p + TL : 2048 * p + T], uvec, csl,
                        op0=OP.mult, op1=OP.add,
                    )
                nc.sync.dma_start(
                    ag2_in.rearrange("(g p) t -> p g t", p=128),
                    agst[:].rearrange("p (g t) -> p g t", g=NP),
                )
                if "coll" not in skip:
                    nc.gpsimd.collective_compute(
                        "AllGather", OP.bypass,
                        replica_groups=[[0, 1], [2, 3], [4, 5], [6, 7]],
                        ins=[ag2_in.opt()], outs=[ag2_out.opt()],
                    )

                # --- h assembly: h = x + y (SBUF bf16); x = 2x + y ---
                hT = kv.tile([128, DC * TL], BF16, tag="KT")
                for c in range(DC):
                    low = c < 4
                    bbc = strm.tile([128, TL], BF16, tag="bbc", bufs=2)
                    nc.sync.dma_start(
                        bbc[:],
                        ag2_out[(0 if low else 512) + 128 * (c % 4) :
                                (0 if low else 512) + 128 * (c % 4) + 128, :],
                    )
                    t1 = strm.tile([128, TL], BF16, tag="t1", bufs=1)
                    nc.vector.tensor_scalar(
                        t1[:],
                        yown[:, 2048 * (c % 4) + (0 if low else TL) :
                             2048 * (c % 4) + (TL if low else T)],
                        coef(0 if low else 1), None, op0=OP.mult,
                    )
                    nc.vector.scalar_tensor_tensor(
                        t1[:], bbc[:], coefo(0 if low else 1), t1[:],
                        op0=OP.mult, op1=OP.add,
                    )
                    xs = xT[:, c * TL : (c + 1) * TL]
                    nc.vector.tensor_tensor(
                        hT[:, c * TL : (c + 1) * TL], xs, t1[:], op=OP.add
                    )
                    nc.vector.scalar_tensor_tensor(
                        xs, xs, 2.0, t1[:], op0=OP.mult, op1=OP.add
                    )

                def h_slice(dc, tt, _h=hT):
                    return _h[:, dc * TL + tt * 512 : dc * TL + tt * 512 + 512]

                xn2 = ln_apply(h_slice, 2 * l + 1, src_16=True, xn_tag="xn")

                # --- MLP ---
                def xn2_sl(dc, tt, _x=xn2):
                    return _x[:, dc * TL + tt * 512 : dc * TL + tt * 512 + 512]

                bm1_sb = sml.tile([128, 32], F32, tag="bm1")
                nc.sync.dma_start(
                    bm1_sb[:],
                    bm1_in.ap()[l : l + 1, :].rearrange("o (c p) -> (o p) c", p=128),
                )
                bm2_sb = sml.tile([128, 8], F32, tag="bm2")
                nc.sync.dma_start(
                    bm2_sb[:],
                    bm2_in.ap()[l : l + 1, :].rearrange("o (c p) -> (o p) c", p=128),
                )
                for tt in range(2 if "mlp" not in skip else 0):
                    zA = kv.tile([128, 16 * 512], BF16, tag="KT")
                    zB = kv.tile([128, 16 * 512], BF16, tag="Vsb")

                    def zsl(j, _zA=zA, _zB=zB):
                        t_ = _zA if j < 16 else _zB
                        return t_[:, (j % 16) * 512 : (j % 16) * 512 + 512]

                    for u in range(16):
                        wt1 = wp.tile([128, 2048], BF16, tag="wt1")
                        nc.sync.dma_start(wt1[:], w1h.ap()[l * 16 + u])
                        z_ps = ps_mm.tile([128, 1024], F32, tag="mm")
                        for dc in range(DC):
                            r_sl = xn2_sl(dc, tt)
                            for j2 in range(2):
                                nc.tensor.matmul(
                                    z_ps[:, j2 * 512 : j2 * 512 + 512],
                                    wt1[:, j2 * 1024 + dc * 128 : j2 * 1024 + dc * 128 + 128],
                                    r_sl,
                                    start=(dc == 0), stop=(dc == DC - 1),
                                )
                        for j2 in range(2):
                            j = 2 * u + j2
                            nc.scalar.activation(
                                zsl(j), z_ps[:, j2 * 512 : j2 * 512 + 512], AF.Relu,
                                bias=bm1_sb[:, j : j + 1], scale=1.0,
                            )
                    for oc in range(8):
                        wt2 = wp.tile([128, 4096], BF16, tag="wt2")
                        nc.sync.dma_start(wt2[:], w2h.ap()[l * 8 + oc])
                        d_ps = ps_mm.tile([128, 512], F32, tag="mm")
                        for j in range(32):
                            nc.tensor.matmul(
                                d_ps[:], wt2[:, j * 128 : j * 128 + 128], zsl(j),
                                start=(j == 0), stop=(j == 31),
                            )
                        xsl = xT[:, oc * TL + tt * 512 : oc * TL + tt * 512 + 512]
                        nc.vector.scalar_tensor_tensor(
                            xsl, d_ps[:], bm2_sb[:, oc : oc + 1], xsl,
                            op0=OP.add, op1=OP.add,
                        )

            # ---------------- final LN + unembed ----------------
            xnf = ln_apply(x_slice, 2 * n_layers, src_16=False, xn_tag="xn")
            bun_sb = sml.tile([V, 1], F32, tag="bun", bufs=1)
            nc.sync.dma_start(bun_sb[:], bun.ap())
            out_sb = att.tile([V, TL], F32, tag="qown")
            for tt in range(2):
                o_ps = ps_mm.tile([64, 512], F32, tag="mm")
                for dc in range(DC):
                    wtu = wp.tile([128, 64], BF16, tag="wtu")
                    nc.sync.dma_start(wtu[:], wunT.ap()[dc * 128 : (dc + 1) * 128, :])
                    nc.tensor.matmul(
                        o_ps[:], wtu[:],
                        xnf[:, dc * TL + tt * 512 : dc * TL + tt * 512 + 512],
                        start=(dc == 0), stop=(dc == DC - 1),
                    )
                nc.scalar.activation(
                    out_sb[:, tt * 512 : tt * 512 + 512], o_ps[:], AF.Identity,
                    bias=bun_sb[:, 0:1], scale=1.0,
                )
            nc.sync.dma_start(outT.ap(), out_sb[:])

    nc.compile()
    return nc


# ---------------- host side ----------------


def prep_inputs(inputs, n_layers=L):
    f32 = np.float32
    bf16 = ml_dtypes.bfloat16
    toks = np.asarray(inputs["toks"], f32)
    W_tok = np.asarray(inputs["W_tok"], f32)
    W_pos = np.asarray(inputs["W_pos"], f32)
    Wqkv = np.asarray(inputs["Wqkv"], f32)
    W1 = np.asarray(inputs["W1"], f32)
    W2 = np.asarray(inputs["W2"], f32)
    Wun = np.asarray(inputs["Wun"], f32)
    bun = np.asarray(inputs["bun"], f32)
    g1, be1 = np.asarray(inputs["g1"], f32), np.asarray(inputs["be1"], f32)
    g2, be2 = np.asarray(inputs["g2"], f32), np.asarray(inputs["be2"], f32)
    gf, bf_ = np.asarray(inputs["gf"], f32), np.asarray(inputs["bf"], f32)
    bm1 = np.asarray(inputs["bm1"], f32)
    bm2 = np.asarray(inputs["bm2"], f32)

    # Fold LN gains into the consuming weights; biases become b' = b/g added
    # to the normalized activations inside the kernel.
    def safediv(b, g):
        return b / np.where(g == 0.0, 1.0, g)

    Wq = Wqkv[:n_layers] * g1[:n_layers, None, :]
    W1f = W1[:n_layers] * g2[:n_layers, None, :]
    Wunf = Wun * gf[None, :]
    bvec = np.zeros((2 * n_layers + 1, D), f32)
    bvec[0 : 2 * n_layers : 2] = safediv(be1[:n_layers], g1[:n_layers])
    bvec[1 : 2 * n_layers : 2] = safediv(be2[:n_layers], g2[:n_layers])
    bvec[2 * n_layers] = safediv(bf_, gf)

    masks = np.zeros((4, 128, 512), np.float32)
    i = np.arange(128)[:, None]
    j = np.arange(512)[None, :]
    for d in range(4):
        masks[d] = (j >= 128 * d + i).astype(f32)
    masks_bf = masks.astype(bf16)
    sel2 = np.zeros((65, 128), f32)
    sel2[0, 0:64] = 1.0
    sel2[64, 64:128] = 1.0
    ones = np.ones((128, 128), f32)

    # W1 lhsT j-pair tiles: w1h[l*16+u][p, j2*1024 + c*128 + o]
    #   = W1f[l, (2u+j2)*128 + o, c*128 + p]
    w1h = np.ascontiguousarray(
        W1f.reshape(n_layers, 16, 2, 128, 8, 128).transpose(0, 1, 5, 2, 4, 3)
    ).reshape(n_layers * 16, 128, 2048).astype(bf16)
    # W2 lhsT per-oc tiles: w2h[l*8+oc][p, j*128 + o] = W2[l, oc*128+o, j*128+p]
    w2h = np.ascontiguousarray(
        W2[:n_layers].reshape(n_layers, 8, 128, 32, 128).transpose(0, 1, 4, 3, 2)
    ).reshape(n_layers * 8, 128, 4096).astype(bf16)
    wunT = np.ascontiguousarray(Wunf.T).astype(bf16)
    wtokT = np.ascontiguousarray(W_tok.T)

    in_maps = []
    for c in range(NC):
        b, jj = c // 2, c % 2
        ho = list(range(8 * jj, 8 * jj + 8)) + list(range(8 * (1 - jj), 8 * (1 - jj) + 8))
        idx_q = np.concatenate([np.arange(192 * h, 192 * h + 64) for h in ho])
        perm = np.concatenate([idx_q, idx_q + 64, idx_q + 128])
        Wqp = Wq[:, perm, :]
        # Q/K lhsT tiles: wqkh[l*16+oc][p, c*128+o] = Wqp[l, oc*128+o, c*128+p]
        wqkh = np.ascontiguousarray(
            Wqp[:, :2048].reshape(n_layers, 16, 128, 8, 128).transpose(0, 1, 4, 3, 2)
        ).reshape(n_layers * 16, 128, 1024).astype(bf16)
        # V rhs tiles: wvh[l*2+oh][p, c*512+o] = Wqp[l, 2048 + oh*512 + o, c*128+p]
        wvh = np.ascontiguousarray(
            Wqp[:, 2048:].reshape(n_layers, 2, 512, 8, 128).transpose(0, 1, 4, 3, 2)
        ).reshape(n_layers * 2, 128, 4096).astype(bf16)
        u = 1.0 if jj == 0 else 0.0
        uv = np.zeros((128, 3), f32)
        uv[:, 0] = u
        uv[:, 1] = 1.0 - u
        uv[:, 2] = EPS
        in_maps.append(
            {
                "toksT": np.ascontiguousarray(toks[b, TL * jj : TL * jj + TL, :].T),
                "posT": np.ascontiguousarray(W_pos[:, TL * jj : TL * jj + TL]),
                "wtokT": wtokT,
                "wqkh": wqkh,
                "wvh": wvh,
                "w1h": w1h,
                "w2h": w2h,
                "bvec": bvec,
                "wunT": wunT,
                "bm1": bm1[:n_layers],
                "bm2": bm2[:n_layers],
                "bun": bun.reshape(V, 1),
                "masks": masks_bf,
                "sel2": sel2,
                "ones": ones,
                "uv": uv,
            }
        )
    return in_maps


def kernel(**inputs):
    if "prog" not in _CACHE:
        _CACHE["prog"] = build_program()
    nc = _CACHE["prog"]
    in_maps = prep_inputs(inputs)
    res = run_bass_kernel_spmd(nc, in_maps, list(range(NC)))
    out = np.zeros((B, T, V), np.float32)
    for c in range(NC):
        b, jj = c // 2, c % 2
        out[b, TL * jj : TL * jj + TL, :] = res.results[c]["outT"].T
    return out
